# revision 24
# baseline (speedup 1.0000x reference)
"""Paged-KV-cache causal GQA attention on 8 TRN2 NeuronCores.

Problem shape (hardcoded): B=8 seqs x S=1024 tokens, H=32 q-heads,
KVH=8 kv-heads (GQA group 4), D=128, block_size=256, 40 cache blocks.

Sharding: data parallel, one sequence per core. Host does the
store_kvcache scatter + block-table gather (layout work) and per-core
layout prep (head-major transposes + bf16 cast, scale folded into q);
each core runs causal flash attention for its sequence over all 32
heads. Softmax denominator division happens on the host: the device
ships unnormalized PV output plus the rowsum column.

Device pipeline per head-pair, SIX score groups per head
(896/384/1024/1024/896/384 cols, each <=2 psum banks; pspool 3-deep so
a group's QK waits on the exp from 1.5 groups back instead of 1 -
enough slack that the PE never stalls on ACT):
  warmup: 48 dummy matmuls bridge the initial DMA wait so the PE HAM
          clock gate stays 8/8 into the first real matmul (~15us).
  QK:     scores^T[k,q] = K^T.T @ Q^T (PE, bf16).
  exp:    one wide op per (head, group); the two 1024 groups of both
          heads + nothing else... precisely G2 both heads and G3 head0
          run bf16-Schraudolph on the DVE (bits = rne(s*184.664 +
          16248.6) as int16, bitcast bf16; rel err ~1.8% RMS, cancels
          in softmax num/denom), the rest on ACT. ACT ~102us / DVE
          ~110us / GPSIMD ~107us, all under the PE.
  mask:   diagonal tiles masked into [128,128] tiles on the
          otherwise-idle GPSIMD (SBUF-only op; DVE for the last pair
          where latency matters).
  PV:     po[q,0:129] += P^T.T @ [V|1] per (q-tile, k-tile), two
          q-tiles per psum bank. Every q-tile pair is emitted exactly
          TWO group-sections after its diagonal's group (cross-pair
          queue), which (a) hides the ~400ns gpsimd mask latency,
          (b) spaces po-bank reuse a full section from its evac.
  evac:   DVE copies the raw [128,258] po pair (128 out cols + rowsum
          col per q-tile) to SBUF as bf16, issued ahead of the exps in
          each section so po turnaround never blocks the PE.
  store:  one [128,1032] store per head on the sync HWDGE ring,
          deferred 4 sections after the head's last evac so the DMA's
          SBUF reads land in the PE-lightest section (the 384-wide
          G5) instead of under a PV burst.
Ramp DMAs use >=512-col pieces on the gpsimd ring (wakes ~2.5us
earlier than sync). Last pair: masks on DVE, qt4 right after G4, and
a short qt5/qt6/qt7 tail after the final exp.
"""

import sys

import numpy as np
import ml_dtypes

sys.path.insert(0, "/opt/trn_rl_repo")

import concourse.bass as bass  # noqa: E402
import concourse.mybir as mybir  # noqa: E402
import concourse.tile as tile  # noqa: E402
from concourse import bacc  # noqa: E402
from concourse.bass_utils import run_bass_kernel_spmd  # noqa: E402

B, S = 8, 1024
H, KVH, D = 32, 8, 128
G = H // KVH
NT = S // 128  # 8 k/q tiles of 128 per sequence
VW = 132  # v tile row: 128 v cols + ones col + pad
SCALE = 1.0 / float(np.sqrt(D))
BF = mybir.dt.bfloat16
F32 = mybir.dt.float32
I16 = mybir.dt.int16
_NC = None

# Score groups: (qc, tw, entries) with entries = ordered matmuls
# (kt, q_tile_off_in_chunk, width, psum_off). No region crosses a
# 512-f32 psum bank boundary.
GROUPS6 = [
    (0, 896, [(0, 0, 512, 0), (1, 1, 384, 512)]),
    (0, 384, [(2, 2, 256, 0), (3, 3, 128, 256)]),
    (1, 1024, [(0, 0, 512, 0), (1, 0, 512, 512)]),
    (1, 1024, [(2, 0, 512, 0), (3, 0, 512, 512)]),
    (1, 896, [(4, 0, 512, 0), (5, 1, 384, 512)]),
    (1, 384, [(6, 2, 256, 0), (7, 3, 128, 256)]),
]
# q-tile pairs finishing (diagonal exp'd+masked) at each group index
PV_FIN = {0: (0, 1), 1: (2, 3), 4: (4, 5), 5: (6, 7)}
PV_DELAY = 2   # sections between a diagonal's group and its PV emission
ST_DELAY = 4   # sections between a head's last evac and its store
# groups whose exp runs as bf16-Schraudolph on the DVE: (gi, head-in-pair).
# qc1 groups only (qc0 has small softmax denominators -> poor error
# cancellation), at most one per group section so the DVE's serial
# [evacs, schrau] chain never delays the pspool rotation by two exps.
SCHRAU = {(2, 0), (3, 1), (4, 1)}
# the first pair's opening QK runs in 256-col pieces so the very first
# matmul needs only kT[:,0:128] + q0[:,0:256] (~96KB) off the ramp
G0_SPLIT = [(0, 0, 256, 0), (0, 2, 256, 256), (1, 1, 384, 512)]


def _build_nc():
    nc = bacc.Bacc("TRN2", target_bir_lowering=False, debug=False, num_devices=8)
    qT = nc.dram_tensor("qT", [H, D, S], BF, kind="ExternalInput").ap()
    kT = nc.dram_tensor("kT", [KVH, D, S], BF, kind="ExternalInput").ap()
    v1 = nc.dram_tensor("v1", [KVH, NT, 128, VW], BF, kind="ExternalInput").ap()
    # per head: 4 q-tile pairs x [128 rows, 2*(128 out + rowsum)]
    out = nc.dram_tensor("out", [H, 4, 128, 258], BF, kind="ExternalOutput").ap()
    mask_np = np.triu(np.ones((128, 128), dtype=ml_dtypes.bfloat16))
    mask_dram = nc.inline_tensor(mask_np, "tri_mask").ap()

    with tile.TileContext(nc) as tc:
        with (
            tc.tile_pool(name="singles", bufs=1) as singles,
            tc.tile_pool(name="qpool", bufs=6) as qpool,
            tc.tile_pool(name="ppool", bufs=14) as ppool,
            tc.tile_pool(name="dpool", bufs=22) as dpool,
            tc.tile_pool(name="opool", bufs=4) as opool,
            tc.tile_pool(name="pspool", bufs=3, space="PSUM") as pspool,
            tc.tile_pool(name="popool", bufs=2, space="PSUM") as popool,
        ):
            # --- HAM warmup: dummy matmuls with no data deps keep the
            # PE busy until the first real QK data lands (~15us) so the
            # clock gate is 8/8 when real work starts ---
            warm_sb = singles.tile([128, 256], BF, name="warm_sb")
            nc.vector.memset(warm_sb, 0.0)
            dummy_ps = popool.tile([128, 258], F32, tag="po", name="dummy_ps")
            for i in range(26):
                nc.tensor.matmul(
                    dummy_ps[:, 0:256], lhsT=warm_sb[:, 0:128], rhs=warm_sb,
                    start=True, stop=True, skip_group_check=True,
                )
            for i in range(8):
                nc.tensor.matmul(
                    dummy_ps[:, 0:128], lhsT=warm_sb[:, 0:128],
                    rhs=warm_sb[:, 0:128],
                    start=True, stop=True, skip_group_check=True,
                )

            mask_sb = singles.tile([128, 128], BF)
            kv_sb = []
            for kvh in range(KVH):
                k_t = singles.tile([128, S], BF, name=f"kT_sb{kvh}", tag=f"kT{kvh}")
                v_t = singles.tile(
                    [128, NT * VW], BF, name=f"v1_sb{kvh}", tag=f"v1{kvh}"
                )
                kv_sb.append((k_t, v_t))

            def load_kv(kvh):
                # kT on the sync HWDGE ring; v1 on the gpsimd SWDGE ring so
                # the two streams' kickoffs and transfers run in parallel
                nc.sync.dma_start(out=kv_sb[kvh][0], in_=kT[kvh])
                nc.gpsimd.dma_start(
                    out=kv_sb[kvh][1].rearrange("p (t c) -> p t c", t=NT),
                    in_=v1[kvh].rearrange("t p c -> p t c"),
                )

            q_tiles = {}

            def load_q(h):
                if h < H and h not in q_tiles:
                    q_tiles[h] = qpool.tile([128, S], BF, tag="q", name=f"q_sb{h}")
                    nc.sync.dma_start(out=q_tiles[h], in_=qT[h])

            # fast start: the head phase is HBM-bandwidth-bound (all 8
            # cores burst-load at once), so the first-matmul-critical
            # bytes ride the gpsimd SWDGE ring (its engine wakes ~2.5us
            # before the sync ring), in >=512-col pieces (descriptor
            # size sets ring bandwidth); v1[0] rides the scalar ring
            q_tiles[0] = qpool.tile([128, S], BF, tag="q", name="q_sb0")
            q_tiles[1] = qpool.tile([128, S], BF, tag="q", name="q_sb1")
            nc.gpsimd.dma_start(out=kv_sb[0][0][:, 0:128], in_=kT[0][:, 0:128])
            nc.gpsimd.dma_start(out=q_tiles[0][:, 0:256], in_=qT[0][:, 0:256])
            nc.gpsimd.dma_start(out=q_tiles[0][:, 256:512], in_=qT[0][:, 256:512])
            nc.gpsimd.dma_start(out=kv_sb[0][0][:, 128:256], in_=kT[0][:, 128:256])
            nc.gpsimd.dma_start(out=mask_sb, in_=mask_dram)
            nc.sync.dma_start(out=q_tiles[1][:, 0:256], in_=qT[1][:, 0:256])
            nc.sync.dma_start(out=q_tiles[1][:, 256:512], in_=qT[1][:, 256:512])
            nc.sync.dma_start(out=kv_sb[0][0][:, 256:512], in_=kT[0][:, 256:512])
            nc.sync.dma_start(out=q_tiles[0][:, 512:], in_=qT[0][:, 512:])
            nc.sync.dma_start(out=q_tiles[1][:, 512:], in_=qT[1][:, 512:])
            nc.sync.dma_start(out=kv_sb[0][0][:, 512:], in_=kT[0][:, 512:])
            nc.scalar.dma_start(
                out=kv_sb[0][1].rearrange("p (t c) -> p t c", t=NT)[:, 0:2, :],
                in_=v1[0].rearrange("t p c -> p t c")[:, 0:2, :],
            )
            nc.scalar.dma_start(
                out=kv_sb[0][1].rearrange("p (t c) -> p t c", t=NT)[:, 2:, :],
                in_=v1[0].rearrange("t p c -> p t c")[:, 2:, :],
            )
            q_tiles[2] = qpool.tile([128, S], BF, tag="q", name="q_sb2")
            q_tiles[3] = qpool.tile([128, S], BF, tag="q", name="q_sb3")
            nc.sync.dma_start(out=q_tiles[2][:, 0:512], in_=qT[2][:, 0:512])
            nc.sync.dma_start(out=q_tiles[3][:, 0:512], in_=qT[3][:, 0:512])
            nc.sync.dma_start(out=q_tiles[2][:, 512:], in_=qT[2][:, 512:])
            nc.sync.dma_start(out=q_tiles[3][:, 512:], in_=qT[3][:, 512:])
            load_kv(1)

            # cross-pair state
            p_loc = {}  # (h, qc, kt) -> (tile, off, qoff)
            d_loc = {}  # (h, qc, kt) -> masked diagonal tile
            po2 = {}    # (h, qt//2) -> psum tile [128, 258]
            osb = {}    # h -> output staging tile [128, 1032]
            pendq = []  # deferred actions: (due_point, kind, h, qt)
            point = [0]  # global emission-point counter (one per group)

            def pv_run(h, qt, start_kt=0, stop_kt=None):
                # accumulate P.T @ [V|1] over qt's k tiles back-to-back;
                # two q-tiles share one psum bank (single start=True per
                # bank)
                qc = qt // 4
                v1_sb = kv_sb[h // G][1]
                if qt % 2 == 0 and start_kt == 0:
                    po2[(h, qt // 2)] = popool.tile(
                        [128, 258], F32, tag="po", name=f"po_{h}_{qt}"
                    )
                po = po2[(h, qt // 2)]
                base = (qt % 2) * 129
                end_kt = qt + 1 if stop_kt is None else stop_kt
                for kt in range(start_kt, end_kt):
                    if kt == qt:
                        lhsT = d_loc[(h, qc, kt)]
                    else:
                        t, off, qoff = p_loc[(h, qc, kt)]
                        j = qt - qc * 4
                        lhsT = t[:, off + (j - qoff) * 128:
                                 off + (j - qoff) * 128 + 128]
                    nc.tensor.matmul(
                        po[:, base: base + 129],
                        lhsT=lhsT,
                        rhs=v1_sb[:, kt * VW: kt * VW + 129],
                        start=(kt == 0 and qt % 2 == 0 and start_kt == 0),
                        stop=(kt == qt),
                        skip_group_check=True,
                    )

            def evac(h, pr):
                # raw bf16 copy of the po pair (out cols + rowsum);
                # host does the softmax division
                nc.vector.tensor_copy(
                    osb[h][:, pr * 258: pr * 258 + 258], po2[(h, pr)]
                )

            def store(h, half):
                # half stores land in the PV-free G4/G5 sections so the
                # DMA's SBUF reads never contend with a PV burst's rhs
                nc.sync.dma_start(
                    out=out[h, 2 * half: 2 * half + 2].rearrange(
                        "a p b -> p a b"
                    ),
                    in_=osb[h][:, half * 516: half * 516 + 516].rearrange(
                        "p (a b) -> p a b", a=2
                    ),
                )

            def emit_due():
                p = point[0]
                for item in [x for x in pendq]:
                    due, kind, h3, qt3 = item
                    if due <= p:
                        pendq.remove(item)
                        if kind == 'st':
                            store(h3, qt3)
                            continue
                        pv_run(h3, qt3)
                        if qt3 % 2 == 1:
                            evac(h3, qt3 // 2)
                            if qt3 == 7:
                                pendq.append((p + ST_DELAY - 1, 'st', h3, 0))
                                pendq.append((p + ST_DELAY, 'st', h3, 1))

            for h0 in range(0, H, 2):
                hs = (h0, h0 + 1)
                last = h0 == H - 2
                kvh = h0 // G
                kT_sb = kv_sb[kvh][0]
                load_q(h0 + 2)
                load_q(h0 + 3)
                if h0 % G == 0 and kvh + 2 < KVH:
                    load_kv(kvh + 2)
                for h in hs:
                    osb[h] = opool.tile([128, 1032], BF, tag="o", name=f"o_{h}")

                for gi, (qc, tw, entries) in enumerate(GROUPS6):
                    if h0 == 0 and gi == 0:
                        entries = G0_SPLIT  # same psum layout, finer DMA waits
                    for h in hs:
                        ps = pspool.tile(
                            [128, 1024], F32, tag="ps", name=f"ps_{h}_{gi}",
                        )
                        for kt, qoff, w, off in entries:
                            nc.tensor.matmul(
                                ps[:, off: off + w],
                                lhsT=kT_sb[:, kt * 128: kt * 128 + 128],
                                rhs=q_tiles[h][
                                    :, qc * 512 + qoff * 128:
                                    qc * 512 + qoff * 128 + w
                                ],
                                start=True, stop=True, skip_group_check=True,
                            )
                        # exps/evacs for this section go behind the PVs on
                        # DVE, so emit due work right after h0's QK
                        if h == h0:
                            emit_due()
                        if (gi, h - h0) in SCHRAU:
                            # bf16-Schraudolph exp on the DVE offloads ACT
                            p_i16 = ppool.tile(
                                [128, tw], I16, tag="p", name=f"p_{h}_{gi}"
                            )
                            nc.vector.tensor_scalar(
                                p_i16, ps[:, 0:tw],
                                184.6644353, 16248.6,
                                mybir.AluOpType.mult, mybir.AluOpType.add,
                            )
                            p_sb = p_i16.bitcast(BF)
                        else:
                            p_sb = ppool.tile(
                                [128, tw], BF, tag="p", name=f"p_{h}_{gi}"
                            )
                            nc.scalar.activation(
                                p_sb, ps[:, 0:tw],
                                mybir.ActivationFunctionType.Exp,
                            )
                        masked = set()
                        for kt, qoff, w, off in entries:
                            p_loc[(h, qc, kt)] = (p_sb, off, qoff)
                            if kt >= qc * 4 and kt not in masked:
                                masked.add(kt)  # diagonal: upper-tri mask
                                dt_ = dpool.tile(
                                    [128, 128], BF, tag="d",
                                    name=f"d_{h}_{gi}_{kt}",
                                )
                                j = kt - qc * 4
                                # slow gpsimd masks are fine mid-stream
                                # (PVs 2 sections late); prompt DVE for
                                # the tail
                                meng = nc.vector if last else nc.gpsimd
                                meng.tensor_mul(
                                    dt_,
                                    p_sb[:, off + (j - qoff) * 128:
                                         off + (j - qoff) * 128 + 128],
                                    mask_sb,
                                )
                                d_loc[(h, qc, kt)] = dt_
                    if gi in PV_FIN and not (last and gi >= 4):
                        for qt3 in PV_FIN[gi]:
                            for h in hs:
                                pendq.append(
                                    (point[0] + PV_DELAY, 'pv', h, qt3)
                                )
                    point[0] += 1
                    if last and gi == 4:
                        # tail setup: qt4 AND qt5 now (their last dep is
                        # G4's DVE mask) so only qt6/qt7 trail the G5 exp
                        for h in hs:
                            pv_run(h, 4)
                            pv_run(h, 5)
                            evac(h, 2)
                            nc.sync.dma_start(
                                out=out[h, 2], in_=osb[h][:, 516: 774],
                            )
                    if last and gi == 5:
                        # tail: qt6/qt7 + per-pair evac/store
                        for h in hs:
                            pv_run(h, 6)
                            pv_run(h, 7)
                            evac(h, 3)
                            nc.gpsimd.dma_start(
                                out=out[h, 3], in_=osb[h][:, 774: 1032],
                            )
                            nc.sync.dma_start(
                                out=out[h, 0:2].rearrange("a p b -> p a b"),
                                in_=osb[h][:, 0:516].rearrange(
                                    "p (a b) -> p a b", a=2
                                ),
                            )
            # drain remaining deferred stores/PVs
            point[0] += 10
            emit_due()

    nc.compile()
    return nc


def _get_nc():
    global _NC
    if _NC is None:
        _NC = _build_nc()
    return _NC


def make_in_maps(q, k, v, k_cache, v_cache, slot_mapping, block_tables):
    nb, bs, kvh, d = k_cache.shape
    # store_kvcache scatter (mirrors reference semantics on host)
    kc = k_cache.reshape(nb * bs, kvh, d).copy()
    vc = v_cache.reshape(nb * bs, kvh, d).copy()
    kc[slot_mapping] = k
    vc[slot_mapping] = v
    b, mb = block_tables.shape
    s = q.shape[0] // b
    pos = np.arange(s)
    slot_grid = block_tables[:, pos // bs] * bs + (pos % bs)  # [B, S]
    kf = kc[slot_grid]  # [B, S, KVH, D]
    vf = vc[slot_grid]
    qb = q.reshape(b, s, H, D)

    bf16 = ml_dtypes.bfloat16
    in_maps = []
    for i in range(b):
        qTi = np.ascontiguousarray(
            qb[i].transpose(1, 2, 0) * np.float32(SCALE)
        ).astype(bf16)
        kTi = np.ascontiguousarray(kf[i].transpose(1, 2, 0)).astype(bf16)
        vh = vf[i].transpose(1, 0, 2).reshape(KVH, NT, 128, D)
        v1i = np.zeros((KVH, NT, 128, VW), dtype=bf16)
        v1i[..., :D] = vh.astype(bf16)
        v1i[..., D] = 1.0
        in_maps.append({"qT": qTi, "kT": kTi, "v1": v1i})
    return in_maps


def kernel(q, k, v, k_cache, v_cache, slot_mapping, block_tables):
    # accept jax or numpy inputs
    q = np.asarray(q)
    k = np.asarray(k)
    v = np.asarray(v)
    k_cache = np.asarray(k_cache)
    v_cache = np.asarray(v_cache)
    slot_mapping = np.asarray(slot_mapping)
    block_tables = np.asarray(block_tables)
    out_dtype = q.dtype
    in_maps = make_in_maps(q, k, v, k_cache, v_cache, slot_mapping, block_tables)
    nc = _get_nc()
    res = run_bass_kernel_spmd(nc, in_maps, core_ids=list(range(8)))
    outs = []
    for i in range(B):
        o4 = np.asarray(res.results[i]["out"]).astype(np.float32)
        # [H, 4 pairs, 128 rows, 2*129] -> softmax division on host
        arr = o4.reshape(H, 4, 128, 2, 129)
        o = arr[..., :128] / arr[..., 128:129]  # [H, 4, 128, 2, 128]
        o = o.transpose(0, 1, 3, 2, 4).reshape(H, S, D)
        outs.append(o.transpose(1, 0, 2))  # [S, H, D]
    return np.concatenate(outs, axis=0).astype(out_dtype)
